# revision 43
# baseline (speedup 1.0000x reference)
"""Trainium2 Bass kernel for nn_LogicGatedSNN.

reference computation:
    w = ternary(synapse_states)            # {-1,0,+1}, threshold 1.0
    current = spike_input @ w.T            # [B, OUT]
    gated = current * (refractory<=0)
    spikes = (0.7*membrane + gated) >= adaptive_threshold

Sharding (8 cores): batch 2-way x out_features 4-way.
Each core: B_shard=4096, OUT_shard=512, K=IN=2048.
The narrow (512-col) weight shard keeps the serial DVE ternarize at
~15us/core; the wide batch shard streams in column-quarters so the
PE's phase-B rounds consume them as they land.

Host marshaling (lossless layout/dtype transforms only):
  - spkP: spike_input shard as fp8e4m3, packed so every DMA line is a
    contiguous 2KB: row (q*8+c)*128+p = [k=2c row p, k=2c+1 row p] of
    column-quarter q
  - wTp: synapse_states shard as fp8e4m3, compare-preserving encode
    (values whose (>1)/(<-1) outcome rounding would flip get bumped past
    the threshold), k-quads packed to 2KB rows; ternarize runs on device
  - nvec: membrane/threshold/refractory in [128, 4] per-partition layout

Device per core:
  - ternarize wTp -> fp8 {-1,0,+1} weights resident in SBUF (2 DVE ops
    per k-quad)
  - phase A: 8 PSUM groups (4 m-tiles x first 2 n-tiles) stay open
    across the k loop, so the PE consumes k-pairs as the DMA+ternarize
    pipeline delivers them
  - phase B: remaining 6 n-tiles, m-outer (group stops staggered so the
    epilogues pipeline behind the PE)
  - 256 fp8 DoubleRow matmuls, fp32 PSUM (exact: integer currents)
  - fused epilogue per psum tile: out_u8 = (current + bias_o) >= thr_o
    bias_o = 0.7*mem normally; +/-1e30 when refractory (always/never
    fire, chosen by the exact reference compare 0.7*mem >= thr)
Output: out_u8 [OUT_shard, B_shard]; host transposes/casts/assembles.
"""
import os
import sys

sys.path.insert(0, "/opt/trn_rl_repo")
_HERE = os.path.dirname(os.path.abspath(__file__))
if _HERE not in sys.path:
    sys.path.insert(0, _HERE)

import numpy as np
import ml_dtypes

from concourse import bass, mybir
from concourse import tile
from concourse.bass_utils import run_bass_kernel_spmd

# ---- walrus CTRL sync-wait-slot workaround (inline, kernel.py must be
# self-contained). The TileContext tail drain carries one SyncWait per
# outstanding proc; this walrus build's CTRL template holds only 1.
import concourse.tile as _tile
from concourse.vector_clock import ScopedClock as _ScopedClock


def _patched_drain_and_barrier(self, tick_clock, wait_clock):
    nc = self.nc
    drain_inst = nc.sync.drain()
    wait_clock.add_sem_waits(
        drain_inst.ins, _ScopedClock({None: tick_clock.global_clock})
    )
    si = drain_inst.ins.sync_info
    if si is not None and si.on_wait and len(si.on_wait) > 1:
        waits = list(si.on_wait)
        si.on_wait = waits[:1]
        for i in range(1, len(waits)):
            extra = nc.sync.drain()
            esi = extra.ins.sync_info
            if esi is None:
                extra.ins.sync_info = mybir.SyncInfo(
                    on_wait=[waits[i]], on_update=[]
                )
            else:
                esi.on_wait = list(esi.on_wait or []) + [waits[i]]
    nc.all_engine_barrier()
    assert self.sems is not None
    popped = nc._tile_sem_poison_stack.pop()
    assert popped is self._sem_poison
    # no trailing all_engine_barrier: the sem-clear instructions still
    # complete before their engine's stream ends, and every engine is
    # already quiesced by the barrier above — saves ~0.7us of teardown
    nc.clear_and_free_semaphores(list(self.sems.allocated().values()))


_tile.TileContext._drain_and_barrier = _patched_drain_and_barrier
# ---- end workaround


def _split_multi_waits(nc, max_waits=1):
    """This walrus build's instruction templates carry at most one
    semaphore wait. Hoist extra waits onto NoOps inserted just before the
    owning instruction on the same engine (engines execute their stream in
    order, so blocking semantics are identical)."""
    ctr = 0
    for f in nc.m.functions:
        for bb in f.blocks:
            new = []
            for inst in bb.instructions:
                si = inst.sync_info
                if si is not None and si.on_wait and len(si.on_wait) > max_waits:
                    waits = list(si.on_wait)
                    extra, keep = waits[:-max_waits], waits[-max_waits:]
                    for i in range(0, len(extra), max_waits):
                        ctr += 1
                        nop = mybir.InstNoOp(
                            name=f"{inst.name}-wsp{ctr}", ins=[], outs=[]
                        )
                        nop.engine = inst.engine
                        nop.bass_nofuse = True
                        nop.sync_info = mybir.SyncInfo(
                            on_wait=extra[i:i + max_waits], on_update=[]
                        )
                        new.append(nop)
                    si.on_wait = keep
                new.append(inst)
            bb.instructions = new


def _install_ntff_shim():
    """Provide antenv.axon_hooks (absent in this container) so
    run_bass_kernel_spmd(trace=True) can capture NTFF profiles via the
    loaded libaxon_pjrt.so C ABI."""
    import types
    import contextlib
    import ctypes

    try:
        from antenv import axon_hooks  # noqa: F401
        return
    except ImportError:
        pass
    so_path = "/opt/axon/libaxon_pjrt.so"
    if not os.path.exists(so_path):
        return
    lib = ctypes.CDLL(so_path)
    if not hasattr(lib, "axon_start_nrt_profile"):
        return
    lib.axon_start_nrt_profile.argtypes = [
        ctypes.POINTER(ctypes.c_int64), ctypes.c_size_t
    ]
    lib.axon_start_nrt_profile.restype = ctypes.c_int64
    lib.axon_stop_nrt_profile.argtypes = [ctypes.c_char_p]
    lib.axon_stop_nrt_profile.restype = ctypes.c_int64

    @contextlib.contextmanager
    def _hook(output_dir, device_ids):
        import jax

        jax.devices()
        if device_ids:
            ids = (ctypes.c_int64 * len(device_ids))(*device_ids)
            rc = lib.axon_start_nrt_profile(ids, len(device_ids))
        else:
            rc = lib.axon_start_nrt_profile(None, 0)
        if rc != 0:
            raise RuntimeError(f"axon_start_nrt_profile rc={rc}")
        try:
            yield
        finally:
            n = lib.axon_stop_nrt_profile(str(output_dir).encode())
            print(f"profile: {n} file(s) -> {output_dir}", file=sys.stderr)

    mod = types.ModuleType("antenv.axon_hooks")
    mod.get_axon_ntff_profile_hook = lambda: _hook
    mod.set_axon_ntff_profile_hook = lambda h: None
    sys.modules["antenv.axon_hooks"] = mod


_install_ntff_shim()

dt = mybir.dt

B, IN, OUT = 8192, 2048, 2048
PB, QO = 2, 4                 # batch blocks x out blocks = 8 cores
BS, OS = B // PB, OUT // QO   # 4096, 512 per-core shard sizes
KT = IN // 128                # 16 k-tiles
KT2 = KT // 2                 # 8 DoubleRow k-pairs
KQ = KT // 4                  # 4 packed k-quads (weight DMA/tern unit)
MT = OS // 128                # 4 m-tiles (out rows per core)
NB = 512                      # moving free dim per matmul
NT = BS // NB                 # 8 n-tiles
NQ = 4                        # spike column-quarters (1024 cols each)
BIG = 1.0e30
WARMUP = 18

LAST_EXEC_TIME_NS = None
LAST_TRACE = None

_BUILT = None


def _build():
    nc = bass.Bass()
    # spike shard packed in column-quarters with 2KB rows:
    # row ((q*KT2 + c)*128 + p) = spk[k=2c, row p, qcols] ++ spk[k=2c+1, ...]
    spkP = nc.dram_tensor("spkP", [NQ * KT2 * 128, 2048], dt.float8e4,
                          kind="ExternalInput")
    # weight k-quads packed to 2KB rows:
    # row (t*128+p) = wT[(4t)*128+p, :] ++ ... ++ wT[(4t+3)*128+p, :]
    wTp = nc.dram_tensor("wTp", [KQ * 128, 4 * OS], dt.float8e4,
                         kind="ExternalInput")
    nvec = nc.dram_tensor("nvec", [128, 3 * MT], dt.float32, kind="ExternalInput")
    out = nc.dram_tensor("out_u8", [OS, BS], dt.uint8, kind="ExternalOutput")

    AO = mybir.AluOpType

    with tile.TileContext(nc) as tc:
        with tc.tile_pool(name="const", bufs=1) as cpool, \
             tc.tile_pool(name="wq", bufs=1) as wqpool, \
             tc.tile_pool(name="spk", bufs=1) as spkpool, \
             tc.tile_pool(name="wf", bufs=4) as wfpool, \
             tc.tile_pool(name="tern", bufs=1) as ternpool, \
             tc.tile_pool(name="outm", bufs=16) as outpool, \
             tc.tile_pool(name="ps", bufs=8, space="PSUM") as pspool:

            # resident ternary weights + spikes (fp8: exact for {0,1}
            # spikes and {-1,0,+1} weights)
            wq = wqpool.tile([128, KT * OS], dt.float8e4)      # 8KB/partition
            # spike layout [q][k][1024]: quarter-q k-tile at offset
            # (q*KT + k)*1024 so each quarter DMA writes contiguous 2KB
            spk = spkpool.tile([128, NQ * KT * 1024], dt.float8e4)  # 64KB/part

            # PE warmup: a few dummy matmuls ramp the HAM clock-gate in the
            # window before the first k-chunk of real data lands
            wrm = cpool.tile([128, 512], dt.float8e4)
            nc.gpsimd.memset(wrm[:], 0.0)
            pswrm = pspool.tile([128, NB], dt.float32, tag="ps")
            for i in range(WARMUP):
                nc.tensor.matmul(
                    pswrm[:], wrm[:, 0:128], wrm[:, 0:512],
                    start=(i == 0), stop=(i == WARMUP - 1),
                )

            # ---- loads + ternarize ---------------------------------------
            # weight quads first (their ternarize is the serial DVE stream
            # pacing phase A); alternate the Sync / GpSimd HWDGE rings
            def spk_push(q, c, eng):
                dst0 = (q * KT + 2 * c) * 1024
                eng.dma_start(
                    spk[:, dst0:dst0 + 2048],
                    spkP[(q * KT2 + c) * 128:(q * KT2 + c + 1) * 128, :],
                )

            # push order tuned to the deadlines: wf0/wf1 lead both queues
            # (the DVE ternarize stream is the pacer), wf2/wf3 go early on
            # the gpsimd ring, quarter-0 spike k-pairs fill in behind for
            # phase A, then the remaining quarters in consumption order
            wf_eng = [nc.sync, nc.gpsimd, nc.gpsimd, nc.gpsimd]
            wf = []
            for t in range(KQ):
                w = wfpool.tile([128, 4 * OS], dt.float8e4, name=f"wf{t}")
                wf.append(w)
            wf_eng[0].dma_start(wf[0][:], wTp[0:128, :])
            wf_eng[1].dma_start(wf[1][:], wTp[128:256, :])
            # spk00/spk01 ride second on each ring: phase A's t0/t1 start
            # is gated by their arrival at the slow early ring rate
            spk_push(0, 0, nc.sync)
            spk_push(0, 1, nc.gpsimd)
            # nvec is tiny (12KB) — early so the per-neuron ops the
            # scheduler interleaves into the DVE stream never block it
            nv = cpool.tile([128, 3 * MT], dt.float32)
            nc.sync.dma_start(nv[:], nvec[:])
            wf_eng[2].dma_start(wf[2][:], wTp[256:384, :])
            wf_eng[3].dma_start(wf[3][:], wTp[384:512, :])
            spk_push(0, 2, nc.sync)
            spk_push(0, 3, nc.gpsimd)
            spk_push(0, 4, nc.sync)
            spk_push(0, 5, nc.gpsimd)
            spk_push(0, 6, nc.sync)
            spk_push(0, 7, nc.gpsimd)
            for q in range(1, NQ):
                for c in range(KT2):
                    spk_push(q, c,
                             nc.sync if (q * KT2 + c) % 2 == 0 else nc.gpsimd)
            # ternarize each k-quad in 2 DVE ops: neg = (s < -1);
            # w = (s > 1) - neg, exact {-1,0,+1} in fp8 (bufs=1 neg tile
            # pins the stream to program order — the scheduler otherwise
            # reorders it and the in-order DVE blocks on late arrivals)
            # first and last quads are split into k-pair halves: the first
            # so phase A's t=0 matmuls start ~1.7us earlier, the last so
            # t=6 unblocks before the full stream finishes
            pieces = [(0, 0, 2 * OS), (0, 2 * OS, 4 * OS)]
            pieces += [(t, 0, 4 * OS) for t in range(1, KQ - 1)]
            pieces += [(KQ - 1, 0, 2 * OS), (KQ - 1, 2 * OS, 4 * OS)]
            for t, lo, hi in pieces:
                neg = ternpool.tile([128, hi - lo], dt.float8e4, tag="neg")
                nc.vector.tensor_scalar(
                    neg[:], wf[t][:, lo:hi], -1.0, None, AO.is_lt
                )
                nc.vector.scalar_tensor_tensor(
                    wq[:, t * 4 * OS + lo:t * 4 * OS + hi],
                    wf[t][:, lo:hi], 1.0, neg[:],
                    AO.is_gt, AO.subtract,
                )

            # ---- per-neuron epilogue scalars (after tern issue so the
            # DVE prioritizes the ternarize stream) -----------------------
            mem = nv[:, 0:MT]
            thr = nv[:, MT:2 * MT]
            refr = nv[:, 2 * MT:3 * MT]

            # b07 lives in the bufs=1 ternarize slot: the WAR dependency on
            # the last STT's neg read forces this op (and its dependents)
            # AFTER the tern stream — the scheduler otherwise slots these
            # between tern pieces, stretching the critical path ~1.2us
            b07 = ternpool.tile([128, MT], dt.float32, name="b07", tag="neg")
            nc.vector.tensor_scalar(b07[:], mem, 0.7, None, AO.mult)
            # cond = (0.7*mem >= thr)  — exact reference compare for
            # refractory neurons (their new_v is exactly 0.7*mem)
            cond = cpool.tile([128, MT], dt.float32)
            nc.vector.tensor_tensor(cond[:], b07[:], thr, AO.is_ge)
            # bigsel = cond*2BIG - BIG  in {-BIG, +BIG}
            bigsel = cpool.tile([128, MT], dt.float32)
            nc.vector.tensor_scalar(bigsel[:], cond[:], 2.0 * BIG, -BIG, AO.mult, AO.add)
            # sel = refractory? (refr > 0)
            sel = cpool.tile([128, MT], dt.float32)
            nc.vector.tensor_scalar(sel[:], refr, 0.0, None, AO.is_gt)
            # bias = b07 + sel * (bigsel - b07)
            dvt = cpool.tile([128, MT], dt.float32)
            nc.vector.tensor_sub(dvt[:], bigsel[:], b07[:])
            nc.vector.tensor_mul(dvt[:], dvt[:], sel[:])
            bias = cpool.tile([128, MT], dt.float32)
            nc.vector.tensor_add(bias[:], b07[:], dvt[:])

            # 3D views pairing adjacent 128-row k-tiles for DoubleRow
            # (contraction index i = (2t+j)*128 + p; both operands use the
            # same (p, j) mapping so the sum is the plain dot product)
            wqv = wq[:].rearrange("p (t o) -> p t o", t=KT)
            spkv = spk[:].rearrange("p (q k u) -> p q k u", q=NQ, k=KT)
            DR = mybir.MatmulPerfMode.DoubleRow

            def moving(t, r):
                # n-tile r = column-half (r%2) of quarter (r//2)
                return spkv[:, r // 2, 2 * t:2 * t + 2,
                            (r % 2) * NB:(r % 2 + 1) * NB]

            def epilogue(r, m, ps):
                # spikes = (current + bias_o) >= thr_o — single fused
                # DVE op from PSUM
                om = outpool.tile([128, NB], dt.uint8, name="om")
                nc.vector.tensor_scalar(
                    om[:], ps[:],
                    bias[:, m:m + 1], thr[:, m:m + 1],
                    AO.add, AO.is_ge,
                )
                # stores all on the sync ring: a store-carrying gpsimd
                # queue costs ~3.5us extra in its teardown drain
                nc.sync.dma_start(
                    out[m * 128:(m + 1) * 128, r * NB:(r + 1) * NB],
                    om[:],
                )

            # ---- phase A: n-tiles 0,1 with 8 open PSUM groups ------------
            # k-pair t only needs quad t//2 ternarized + quarter-0 spikes,
            # so the PE streams behind the DMA+DVE pipeline.
            pst = [
                pspool.tile([128, NB], dt.float32, name=f"psA_g{g}", tag="ps")
                for g in range(2 * MT)
            ]
            for t in range(KT2):
                for g in range(2 * MT):
                    m, r = g // 2, g % 2
                    nc.tensor.matmul(
                        pst[g][:],
                        wqv[:, 2 * t:2 * t + 2, m * 128:(m + 1) * 128],
                        moving(t, r),
                        start=(t == 0),
                        stop=(t == KT2 - 1),
                        perf_mode=DR,
                    )
            for g in range(2 * MT):
                epilogue(g % 2, g // 2, pst[g])

            # ---- phase B: n-tiles 2..7, m-outer ---------------------------
            for r in range(2, NT):
                for m in range(MT):
                    ps = pspool.tile([128, NB], dt.float32,
                                     name=f"ps_r{r}_m{m}", tag="ps")
                    for t in range(KT2):
                        nc.tensor.matmul(
                            ps[:],
                            wqv[:, 2 * t:2 * t + 2, m * 128:(m + 1) * 128],
                            moving(t, r),
                            start=(t == 0),
                            stop=(t == KT2 - 1),
                            perf_mode=DR,
                        )
                    epilogue(r, m, ps)

    _split_multi_waits(nc)
    return nc


def _get_built():
    global _BUILT
    if _BUILT is None:
        _BUILT = _build()
    return _BUILT


def kernel(spike_input, synapse_states, membrane_potential,
           adaptive_threshold, refractory_count):
    global LAST_EXEC_TIME_NS, LAST_TRACE
    nc = _get_built()

    spikeT = np.ascontiguousarray(spike_input.astype(ml_dtypes.float8_e4m3).T)
    # compare-preserving 8-bit weight encoding: round f32 to fp8e4m3, then
    # bump the few values whose (>1)/(<-1) outcome rounding would flip.
    # The device ternarize compare sees identical outcomes per element.
    s32 = np.ascontiguousarray(synapse_states.astype(np.float32, copy=False).T)
    t8 = s32.astype(ml_dtypes.float8_e4m3)
    one = ml_dtypes.float8_e4m3(1.0)
    bump = ml_dtypes.float8_e4m3(1.125)
    t8[(s32 > 1.0) & (t8 <= one)] = bump
    t8[(s32 < -1.0) & (t8 >= -one)] = -bump
    wTall = t8  # [IN, OUT] fp8, compare-preserving
    mem = np.asarray(membrane_potential, np.float32)
    thr = np.asarray(adaptive_threshold, np.float32)
    refr = np.asarray(refractory_count, np.float32)

    in_maps = []
    for c in range(PB * QO):
        bi, oj = divmod(c, QO)
        nvec = np.concatenate(
            [
                mem[oj * OS:(oj + 1) * OS].reshape(MT, 128).T,
                thr[oj * OS:(oj + 1) * OS].reshape(MT, 128).T,
                refr[oj * OS:(oj + 1) * OS].reshape(MT, 128).T,
            ],
            axis=1,
        )
        # spike quarter-pack: row ((q*8+c)*128+p) =
        #   [k=2c row p, k=2c+1 row p] of column-quarter q
        sshard = spikeT[:, bi * BS:(bi + 1) * BS]          # [2048, 4096]
        spkP = (sshard.reshape(KT2, 2, 128, NQ, 1024)
                .transpose(3, 0, 2, 1, 4)
                .reshape(NQ * KT2 * 128, 2048))
        # weight quad-pack: row (t*128+p) = wT[(4t..4t+3)*128+p, :]
        wshard = wTall[:, oj * OS:(oj + 1) * OS]           # [2048, 512]
        wTp = (wshard.reshape(KQ, 4, 128, OS)
               .transpose(0, 2, 1, 3)
               .reshape(KQ * 128, 4 * OS))
        in_maps.append({
            "spkP": np.ascontiguousarray(spkP),
            "wTp": np.ascontiguousarray(wTp),
            "nvec": np.ascontiguousarray(nvec).astype(np.float32),
        })

    trace = bool(os.environ.get("KERNEL_PROFILE"))
    res = run_bass_kernel_spmd(
        nc, in_maps, core_ids=list(range(PB * QO)), trace=trace
    )
    LAST_EXEC_TIME_NS = res.exec_time_ns
    LAST_TRACE = getattr(res, "instructions_and_trace", None)

    spikes = np.empty((B, OUT), np.float32)
    for c in range(PB * QO):
        bi, oj = divmod(c, QO)
        spikes[bi * BS:(bi + 1) * BS, oj * OS:(oj + 1) * OS] = \
            res.results[c]["out_u8"].T
    return spikes
